# revision 15
# baseline (speedup 1.0000x reference)
"""Distributed Bass kernel for nn_AllLoss: YOLACT-style loss over 8 cores.

Per-core (one image each):
  cls:  -ln(p[pos]).sum()/K/K  +  -ln(1-p[neg]).sum()/3K/K
  loc:  smooth_l1(pr - encode(gt, anchor)).sum()/K
  msk:  BCE(sigmoid(coef@proto), goalmask).mean(hw).sum(k)/K
        = [ sum softplus(z) - sum_k <y_gk, z_k> ] / 16384 / K
        with  sum_k <y_gk, z_k> = <c_agg, G>,  G[b,p] = <y_b, proto_p>,
              c_agg = onehot(gt_idx)^T @ coef_gathered

Engine plan:
  ACT: one Softplus pass over z (16 rounds of [128,1600] from PSUM) with
       per-round accumulator -> macc.  Single act table (softplus_and_others),
       warmed by a dummy op at t0.  No exp/ln chain.
  PE:  z = protoN^T @ coefT (128 bf16 matmuls, [128,200] each);
       G via 128 accumulated fp32 matmuls on host-transposed mask/proto
       layouts; c_agg onehot matmuls; final partition-reduce.
  DVE: all small losses.  ln() computed with an exponent/mantissa bit split
       and a deg-5 Horner polynomial (no ACT table switch).
  No collective: each core writes its per-image partial (already /N); the
  host sums the 8 scalars as the unshard step.
"""
import sys

sys.path.insert(0, "/opt/trn_rl_repo")
import numpy as np
from concourse import bacc, mybir, tile
from concourse.masks import make_identity

# Pin the single activation table we use (Exp + Ln live together here);
# left alone, the table-load pass may thrash between sets.
_orig_gat = bacc.get_activation_tables


def _gat_one_set(arch):
    t = _orig_gat(arch)
    keep = "natural_log_exp_and_others"
    return {k: (v if k == keep else set()) for k, v in t.items()}


bacc.get_activation_tables = _gat_one_set

N, A, K, B, P, HW = 8, 16368, 200, 20, 4, 128
HW2 = HW * HW  # 16384
KN = 3 * K  # 600
F32 = mybir.dt.float32
BF16 = mybir.dt.bfloat16
I32 = mybir.dt.int32
AF = mybir.ActivationFunctionType
ALU = mybir.AluOpType

# weights fold the final /8 mean over cores
W_POS = -1.0 / (K * K * N)        # stats hold +ln(p)
W_NEG = -1.0 / (KN * K * N)       # stats hold +ln(1-p)
W_LOC = 1.0 / (K * N)
W_S1 = 1.0 / (HW2 * K * N)
W_S2 = -W_S1
INV_LN10 = float(1.0 / np.log(10.0))
LN2 = float(np.log(2.0))
# deg-5 fit of ln(m) on [1,2), max abs err ~1e-5
LNC = [0.030449, -0.28382685, 1.11609003, -2.44002976, 3.5140873, -1.93675974]

ZSLOTS = 8
ZROUNDS = HW // ZSLOTS  # 16


def build_kernel():
    nc = bacc.Bacc(None, target_bir_lowering=False, debug=False)

    big = nc.declare_dram_parameter("big", [A, 13], F32, isOutput=False)
    cls = nc.declare_dram_parameter("cls", [A, 1], F32, isOutput=False)
    proto_z = nc.declare_dram_parameter("proto_z", [128, P * HW2 // 128], F32,
                                        isOutput=False)
    proto_g = nc.declare_dram_parameter("proto_g", [128, P * HW2 // 128], F32,
                                        isOutput=False)
    masks_g = nc.declare_dram_parameter("masks_g", [128, B * HW2 // 128], F32,
                                        isOutput=False)
    gtb = nc.declare_dram_parameter("gtb", [B, 4], F32, isOutput=False)
    # packed indices [128, 9]: c0 pos[:128], c1 pos[128:]+pad, c2 gt[:128],
    # c3 gt[128:]+pad, c4:9 neg (120 rows per col)
    idx = nc.declare_dram_parameter("idx", [128, 9], I32, isOutput=False)
    out = nc.declare_dram_parameter("out", [1, 1], F32, isOutput=True)

    PF_COLS = P * HW2 // 128  # 512
    MG_COLS = B * HW2 // 128  # 2560

    with tile.TileContext(nc) as tc:
        with tc.tile_pool(name="sb", bufs=1) as sb:
            # ---------------- constants / accumulators ----------------
            ident = sb.tile([128, 128], F32)
            make_identity(nc, ident[:])
            ones = sb.tile([128, 1], F32)
            nc.vector.memset(ones[:], 1.0)
            iota_i = sb.tile([128, B], I32)
            nc.gpsimd.iota(iota_i[:], pattern=[[1, B]], base=0,
                           channel_multiplier=0)
            iota_f = sb.tile([128, B], F32)
            nc.vector.tensor_copy(out=iota_f[:], in_=iota_i[:])
            stats = sb.tile([128, 8], F32)
            nc.vector.memset(stats[:], 0.0)
            macc = sb.tile([128, 4], F32)
            nc.vector.memset(macc[:], 0.0)
            # dummy exp so the act table loads during staging
            warm = sb.tile([1, 2], F32)
            nc.vector.memset(warm[:], 0.0)
            nc.scalar.activation(warm[0:1, 1:2], warm[0:1, 0:1], AF.Exp)

            # ---------------- small DMAs (sync queue, priority order) ------
            idxt = sb.tile([128, 9], I32)
            nc.sync.dma_start(out=idxt[:], in_=idx[:, :])
            pf = sb.tile([128, PF_COLS], F32)
            nc.sync.dma_start(out=pf[:], in_=proto_z[:, :])
            posi1 = idxt[:, 0:1]
            posi2 = idxt[0:72, 1:2]
            gti1 = idxt[:, 2:3]
            gti2 = idxt[0:72, 3:4]

            # ---------------- gathers (gpsimd; coef path first) ----------
            bigg1 = sb.tile([128, 13], F32)
            bigg2 = sb.tile([72, 13], F32)
            big_g1 = nc.gpsimd.indirect_dma_start(
                out=bigg1[:], out_offset=None, in_=big[:, :],
                in_offset=bacc.bass.IndirectOffsetOnAxis(ap=posi1, axis=0))
            big_g2 = nc.gpsimd.indirect_dma_start(
                out=bigg2[:], out_offset=None, in_=big[:, :],
                in_offset=bacc.bass.IndirectOffsetOnAxis(ap=posi2, axis=0))

            # ---------------- proto to bf16 [P, HW2] ----------------
            pb = sb.tile([128, PF_COLS], BF16)
            nc.vector.tensor_copy(out=pb[:], in_=pf[:])
            protoNb = sb.tile([P, HW2], BF16)
            nc.sync.dma_start(out=protoNb[:], in_=pb[:])

            # remaining gathers (behind bigg on the gpsimd queue)
            gtg1 = sb.tile([128, 4], F32)
            gtg2 = sb.tile([72, 4], F32)
            nc.gpsimd.indirect_dma_start(
                out=gtg1[:], out_offset=None, in_=gtb[:, :],
                in_offset=bacc.bass.IndirectOffsetOnAxis(ap=gti1, axis=0))
            nc.gpsimd.indirect_dma_start(
                out=gtg2[:], out_offset=None, in_=gtb[:, :],
                in_offset=bacc.bass.IndirectOffsetOnAxis(ap=gti2, axis=0))
            negp = sb.tile([120, 5], F32)
            for j in range(5):
                nc.gpsimd.indirect_dma_start(
                    out=negp[:, j:j + 1], out_offset=None, in_=cls[:, :],
                    in_offset=bacc.bass.IndirectOffsetOnAxis(
                        ap=idxt[0:120, 4 + j:5 + j], axis=0))
            # G operands: straight loads, needed only at the tail; cheap
            # Pool-queue dispatch keeps them off the sync queue.  Pin their
            # dispatch behind the critical gathers so their descriptors don't
            # back up the DMA engines in front of bigg1/bigg2.
            protoG = sb.tile([128, PF_COLS], F32)
            pg_dma = nc.gpsimd.dma_start(out=protoG[:], in_=proto_g[:, :])
            masksG = sb.tile([128, MG_COLS], F32)
            mg_dma = nc.gpsimd.dma_start(out=masksG[:], in_=masks_g[:, :])
            tile.add_dep_helper(pg_dma.ins, big_g2.ins, sync=False,
                                reason="G loads after critical gathers")
            tile.add_dep_helper(mg_dma.ins, big_g2.ins, sync=False,
                                reason="G loads after critical gathers")
            # bf16 copies for single-pass G matmuls
            protoGb = sb.tile([128, PF_COLS], BF16)
            nc.vector.tensor_copy(out=protoGb[:], in_=protoG[:])
            masksGb = sb.tile([128, MG_COLS], BF16)
            nc.vector.tensor_copy(out=masksGb[:], in_=masksG[:])

            # ---------------- coefT (critical path to z matmuls) ----------
            with tc.tile_pool(name="psA", bufs=1, space="PSUM") as psA:
                ctps = psA.tile([P, 256], F32)
                nc.tensor.transpose(out=ctps[:, 0:128], in_=bigg1[:, 4:8],
                                    identity=ident[:])
                nc.tensor.transpose(out=ctps[:, 128:200], in_=bigg2[:, 4:8],
                                    identity=ident[0:72, 0:72])
                coefTb = sb.tile([P, K], BF16)
                nc.scalar.copy(out=coefTb[:], in_=ctps[:, 0:200])

            # ---------------- bulk: z matmuls + exp/product-of-8/ln -------
            # sum softplus(z) = ln prod (1+e^z); products of 8 on DVE in
            # bf16, one batched Ln+accumulate per 4 rounds on ACT.
            scr = sb.tile([128, 8 * K], F32)
            z_mms = []
            sp_instrs = []
            with tc.tile_pool(name="psZ", bufs=2, space="PSUM") as psZ, \
                 tc.tile_pool(name="sb2", bufs=3) as sb2:
                vbuf = None
                for r in range(ZROUNDS):
                    zp = psZ.tile([128, ZSLOTS, 256], F32, tag="zp", name="zp")
                    for s in range(ZSLOTS):
                        t = r * ZSLOTS + s
                        mm = nc.tensor.matmul(
                            out=zp[:, s, 0:K],
                            lhsT=protoNb[:, t * 128:(t + 1) * 128],
                            rhs=coefTb[:], start=True, stop=True)
                        z_mms.append(mm)
                    if r % 4 == 0:
                        vbuf = sb2.tile([128, 8, K], BF16, tag="vbuf",
                                        name="vbuf")
                    et = sb2.tile([128, ZSLOTS * K], BF16, tag="et", name="et")
                    sp = nc.scalar.activation(et[:], zp[:, :, 0:K], AF.Exp)
                    sp_instrs.append(sp)
                    nc.vector.tensor_scalar_add(et[:], et[:], 1.0)
                    t8 = sb2.tile([128, 4 * K], BF16, tag="t8", name="t8")
                    nc.vector.tensor_tensor(out=t8[:], in0=et[:, 0:4 * K],
                                            in1=et[:, 4 * K:8 * K],
                                            op=ALU.mult)
                    nc.vector.tensor_tensor(
                        out=vbuf[:, 2 * (r % 4):2 * (r % 4) + 2, :],
                        in0=t8[:, 0:2 * K], in1=t8[:, 2 * K:4 * K],
                        op=ALU.mult)
                    if r % 4 == 3:
                        nc.scalar.activation(scr[:], vbuf[:], AF.Ln,
                                             accum_out=macc[:, r // 4:r // 4 + 1])

                # ---------------- small losses, all on DVE ----------------
                # lnL layout [128,15]: 0 p1 | 1:3 ahw1 | 3:5 gt1 | 5 p2 |
                #   6:8 ahw2 | 8:10 gt2 | 10:15 (1-pneg)
                lnL = sb.tile([128, 15], F32)
                small = []
                small.append(nc.vector.memset(lnL[:], 1.0))
                small.append(nc.vector.tensor_copy(out=lnL[:, 0:1],
                                                   in_=bigg1[:, 12:13]))
                small.append(nc.vector.tensor_copy(out=lnL[:, 1:3],
                                                   in_=bigg1[:, 10:12]))
                small.append(nc.vector.tensor_copy(out=lnL[:, 3:5],
                                                   in_=gtg1[:, 2:4]))
                small.append(nc.vector.tensor_copy(out=lnL[0:72, 5:6],
                                                   in_=bigg2[:, 12:13]))
                small.append(nc.vector.tensor_copy(out=lnL[0:72, 6:8],
                                                   in_=bigg2[:, 10:12]))
                small.append(nc.vector.tensor_copy(out=lnL[0:72, 8:10],
                                                   in_=gtg2[:, 2:4]))
                small.append(nc.vector.tensor_scalar(
                    out=lnL[0:120, 10:15], in0=negp[:], scalar1=-1.0,
                    scalar2=1.0, op0=ALU.mult, op1=ALU.add))
                # ln via exponent/mantissa split + Horner
                bits = lnL[:].bitcast(I32)
                eI = sb.tile([128, 15], I32)
                small.append(nc.vector.tensor_scalar(
                    out=eI[:], in0=bits, scalar1=23, scalar2=None,
                    op0=ALU.logical_shift_right))
                eF = sb.tile([128, 15], F32)
                small.append(nc.vector.tensor_copy(out=eF[:], in_=eI[:]))
                eT = sb.tile([128, 15], F32)
                small.append(nc.vector.tensor_scalar(
                    out=eT[:], in0=eF[:], scalar1=LN2, scalar2=-127.0 * LN2,
                    op0=ALU.mult, op1=ALU.add))
                mI = sb.tile([128, 15], I32)
                small.append(nc.vector.tensor_scalar(
                    out=mI[:], in0=bits, scalar1=0x007FFFFF, scalar2=0x3F800000,
                    op0=ALU.bitwise_and, op1=ALU.bitwise_or))
                mant = mI[:].bitcast(F32)
                h = sb.tile([128, 15], F32)
                small.append(nc.vector.tensor_scalar(
                    out=h[:], in0=mant, scalar1=LNC[0], scalar2=LNC[1],
                    op0=ALU.mult, op1=ALU.add))
                for c in LNC[2:]:
                    small.append(nc.vector.tensor_tensor(
                        out=h[:], in0=h[:], in1=mant, op=ALU.mult))
                    small.append(nc.vector.tensor_scalar_add(h[:], h[:], float(c)))
                small.append(nc.vector.tensor_tensor(
                    out=h[:], in0=h[:], in1=eT[:], op=ALU.add))
                # h now = ln of every lnL column
                # cls
                small.append(nc.vector.tensor_scalar(
                    out=stats[0:128, 0:1], in0=h[:, 0:1], scalar1=W_POS,
                    scalar2=None, op0=ALU.mult))
                small.append(nc.vector.tensor_scalar(
                    out=stats[0:72, 1:2], in0=h[0:72, 5:6], scalar1=W_POS,
                    scalar2=None, op0=ALU.mult))
                negred = sb.tile([120, 1], F32)
                small.append(nc.vector.tensor_reduce(
                    out=negred[:], in_=h[0:120, 10:15],
                    axis=mybir.AxisListType.X, op=ALU.add))
                small.append(nc.vector.tensor_scalar(
                    out=stats[0:120, 2:3], in0=negred[:], scalar1=W_NEG,
                    scalar2=None, op0=ALU.mult))
                # loc: big cols 0:4 pr, 8:10 ac, 10:12 ahw
                for ci, (bigg, gtg, q, acol, lo) in enumerate(
                        [(bigg1, gtg1, 128, 3, 0), (bigg2, gtg2, 72, 4, 5)]):
                    inv = sb.tile([128, 2], F32, tag=f"inv{ci}", name=f"inv{ci}")
                    small.append(nc.vector.reciprocal(inv[0:q, :], bigg[:, 10:12]))
                    d = sb.tile([128, 4], F32, tag=f"d{ci}", name=f"d{ci}")
                    small.append(nc.vector.tensor_tensor(
                        out=d[0:q, 0:2], in0=gtg[:, 0:2], in1=bigg[:, 8:10],
                        op=ALU.subtract))
                    small.append(nc.vector.tensor_tensor(
                        out=d[0:q, 0:2], in0=d[0:q, 0:2], in1=inv[0:q, :],
                        op=ALU.mult))
                    small.append(nc.vector.tensor_tensor(
                        out=d[0:q, 0:2], in0=bigg[:, 0:2], in1=d[0:q, 0:2],
                        op=ALU.subtract))
                    dln = sb.tile([128, 2], F32, tag=f"dln{ci}", name=f"dln{ci}")
                    small.append(nc.vector.tensor_tensor(
                        out=dln[0:q, :], in0=h[0:q, lo + 3:lo + 5],
                        in1=h[0:q, lo + 1:lo + 3], op=ALU.subtract))
                    small.append(nc.vector.tensor_scalar(
                        out=dln[0:q, :], in0=dln[0:q, :], scalar1=-INV_LN10,
                        scalar2=None, op0=ALU.mult))
                    small.append(nc.vector.tensor_tensor(
                        out=d[0:q, 2:4], in0=bigg[:, 2:4], in1=dln[0:q, :],
                        op=ALU.add))
                    nd = sb.tile([128, 4], F32, tag=f"nd{ci}", name=f"nd{ci}")
                    small.append(nc.vector.tensor_scalar(
                        out=nd[0:q, :], in0=d[0:q, :], scalar1=-1.0,
                        scalar2=None, op0=ALU.mult))
                    ad = sb.tile([128, 4], F32, tag=f"ad{ci}", name=f"ad{ci}")
                    small.append(nc.vector.tensor_tensor(
                        out=ad[0:q, :], in0=d[0:q, :], in1=nd[0:q, :],
                        op=ALU.max))
                    m = sb.tile([128, 4], F32, tag=f"m{ci}", name=f"m{ci}")
                    small.append(nc.vector.tensor_scalar(
                        out=m[0:q, :], in0=ad[0:q, :], scalar1=1.0,
                        scalar2=None, op0=ALU.min))
                    sq = sb.tile([128, 4], F32, tag=f"sq{ci}", name=f"sq{ci}")
                    small.append(nc.vector.tensor_tensor(
                        out=sq[0:q, :], in0=m[0:q, :], in1=m[0:q, :],
                        op=ALU.mult))
                    small.append(nc.vector.tensor_scalar(
                        out=sq[0:q, :], in0=sq[0:q, :], scalar1=0.5,
                        scalar2=None, op0=ALU.mult))
                    small.append(nc.vector.tensor_tensor(
                        out=ad[0:q, :], in0=ad[0:q, :], in1=m[0:q, :],
                        op=ALU.subtract))
                    small.append(nc.vector.tensor_tensor(
                        out=sq[0:q, :], in0=sq[0:q, :], in1=ad[0:q, :],
                        op=ALU.add))
                    red = sb.tile([128, 1], F32, tag=f"red{ci}", name=f"red{ci}")
                    small.append(nc.vector.tensor_reduce(
                        out=red[0:q, :], in_=sq[0:q, :],
                        axis=mybir.AxisListType.X, op=ALU.add))
                    small.append(nc.vector.tensor_scalar(
                        out=stats[0:q, acol:acol + 1], in0=red[0:q, :],
                        scalar1=W_LOC, scalar2=None, op0=ALU.mult))

                # keep the small DVE chain out of the bulk ramp-up
                for si in small:
                    tile.add_dep_helper(si.ins, sp_instrs[2].ins, sync=False,
                                        reason="smalls after bulk start")

                # ---------------- G = masks @ proto^T on PE ----------------
                Gps = psZ.tile([B, P], F32, tag="zp", name="Gps")
                g_mms = []
                for j in range(HW):
                    gm = nc.tensor.matmul(
                        out=Gps[:], lhsT=masksGb[:, j * B:(j + 1) * B],
                        rhs=protoGb[:, j * P:(j + 1) * P],
                        start=(j == 0), stop=(j == HW - 1))
                    g_mms.append(gm)
                tile.add_dep_helper(g_mms[0].ins, z_mms[-1].ins, sync=False,
                                    reason="G after z stream")

                # ---------------- c_agg then S2 = <c_agg, G> --------------
                gidx1 = sb.tile([128, 1], F32)
                gidx2 = sb.tile([72, 1], F32)
                nc.vector.tensor_copy(out=gidx1[:], in_=gti1)
                nc.vector.tensor_copy(out=gidx2[:], in_=gti2)
                H1 = sb.tile([128, B], F32)
                H2 = sb.tile([72, B], F32)
                nc.vector.tensor_scalar(out=H1[:], in0=iota_f[:],
                                        scalar1=gidx1[:, :1], scalar2=None,
                                        op0=ALU.is_equal)
                nc.vector.tensor_scalar(out=H2[:], in0=iota_f[0:72, :],
                                        scalar1=gidx2[:, :1], scalar2=None,
                                        op0=ALU.is_equal)
                cagg = psZ.tile([B, P], F32, tag="zp", name="cagg")
                cm1 = nc.tensor.matmul(out=cagg[:], lhsT=H1[:],
                                       rhs=bigg1[:, 4:8], start=True, stop=False)
                nc.tensor.matmul(out=cagg[:], lhsT=H2[:], rhs=bigg2[:, 4:8],
                                 start=False, stop=True)
                tile.add_dep_helper(cm1.ins, z_mms[-1].ins, sync=False,
                                    reason="cagg after z stream")
                cagg_sb = sb.tile([B, P], F32)
                nc.vector.tensor_copy(out=cagg_sb[:], in_=cagg[:])
                s2 = sb.tile([B, P], F32)
                nc.vector.tensor_tensor(out=s2[:], in0=cagg_sb[:], in1=Gps[:],
                                        op=ALU.mult)
                s2r = sb.tile([B, 1], F32)
                nc.vector.tensor_reduce(out=s2r[:], in_=s2[:],
                                        axis=mybir.AxisListType.X, op=ALU.add)
                nc.vector.tensor_scalar(out=stats[0:B, 5:6], in0=s2r[:],
                                        scalar1=W_S2, scalar2=None, op0=ALU.mult)

                # ---------------- final combine ----------------
                m1 = sb.tile([128, 1], F32)
                nc.vector.tensor_reduce(out=m1[:], in_=macc[:],
                                        axis=mybir.AxisListType.X, op=ALU.add)
                nc.vector.tensor_scalar(out=stats[:, 6:7], in0=m1[:],
                                        scalar1=W_S1, scalar2=None, op0=ALU.mult)
                total = sb.tile([128, 1], F32)
                nc.vector.tensor_reduce(out=total[:], in_=stats[:],
                                        axis=mybir.AxisListType.X, op=ALU.add)
                totps = psZ.tile([1, 1], F32, tag="zp", name="totps")
                nc.tensor.matmul(out=totps[:], lhsT=total[:], rhs=ones[:],
                                 start=True, stop=True)
                fin = sb.tile([1, 1], F32)
                nc.vector.tensor_copy(out=fin[:], in_=totps[0:1, 0:1])
                nc.sync.dma_start(out=out[:, :], in_=fin[:])

    nc.finalize()
    return nc


def _pack_idx(pos, neg, gt):
    m = np.zeros((128, 9), dtype=np.int32)
    m[:, 0] = pos[0:128]
    m[0:72, 1] = pos[128:200]
    m[:, 2] = gt[0:128]
    m[0:72, 3] = gt[128:200]
    for j in range(5):
        m[0:120, 4 + j] = neg[j * 120:(j + 1) * 120]
    return m


def make_in_maps(map_class, map_box, map_coef, proto, anchor_center, anchor_hw,
                 gt_boxes, gt_masks, pos_idx, neg_idx, gt_idx):
    in_maps = []
    for i in range(N):
        big = np.concatenate(
            [map_box[i], map_coef[i], anchor_center, anchor_hw,
             map_class[i].reshape(A, 1)], axis=1).astype(np.float32)
        in_maps.append(dict(
            big=np.ascontiguousarray(big),
            cls=np.ascontiguousarray(map_class[i].reshape(A, 1)),
            proto_z=np.ascontiguousarray(
                proto[i].reshape(128, P * HW2 // 128)),
            proto_g=np.ascontiguousarray(
                proto[i].transpose(2, 1, 0).reshape(128, P * HW2 // 128)),
            masks_g=np.ascontiguousarray(
                gt_masks[i].transpose(2, 1, 0).reshape(128, B * HW2 // 128)),
            gtb=np.ascontiguousarray(gt_boxes[i]),
            idx=_pack_idx(pos_idx[i], neg_idx[i], gt_idx[i]),
        ))
    return in_maps


def kernel(**inputs):
    from concourse.bass_utils import run_bass_kernel_spmd
    nc = build_kernel()
    in_maps = make_in_maps(**inputs)
    res = run_bass_kernel_spmd(nc, in_maps, core_ids=list(range(N)))
    return np.float32(sum(float(res.results[c]["out"][0, 0]) for c in range(N)))


# revision 18
# speedup vs baseline: 1.1153x; 1.1153x over previous
"""Distributed Bass kernel for nn_AllLoss: YOLACT-style loss over 8 cores.

Per-core (one image each):
  cls:  -ln(p[pos]).sum()/K/K  +  -ln(1-p[neg]).sum()/3K/K
  loc:  smooth_l1(pr - encode(gt, anchor)).sum()/K
  msk:  BCE(sigmoid(coef@proto), goalmask).mean(hw).sum(k)/K
        = [ sum softplus(z) - sum_k <y_gk, z_k> ] / 16384 / K
        with  sum_k <y_gk, z_k> = <c_agg, G>,  G[b,p] = <y_b, proto_p>,
              c_agg = onehot(gt_idx)^T @ coef_gathered

Engine plan:
  ACT: one Softplus pass over z (16 rounds of [128,1600] from PSUM) with
       per-round accumulator -> macc.  Single act table (softplus_and_others),
       warmed by a dummy op at t0.  No exp/ln chain.
  PE:  z = protoN^T @ coefT (128 bf16 matmuls, [128,200] each);
       G via 128 accumulated fp32 matmuls on host-transposed mask/proto
       layouts; c_agg onehot matmuls; final partition-reduce.
  DVE: all small losses.  ln() computed with an exponent/mantissa bit split
       and a deg-5 Horner polynomial (no ACT table switch).
  No collective: each core writes its per-image partial (already /N); the
  host sums the 8 scalars as the unshard step.
"""
import sys

sys.path.insert(0, "/opt/trn_rl_repo")
import numpy as np
from concourse import bacc, mybir, tile
from concourse.masks import make_identity

# Pin the single activation table we use (Exp + Ln live together here);
# left alone, the table-load pass may thrash between sets.
_orig_gat = bacc.get_activation_tables


def _gat_one_set(arch):
    t = _orig_gat(arch)
    keep = "natural_log_exp_and_others"
    return {k: (v if k == keep else set()) for k, v in t.items()}


bacc.get_activation_tables = _gat_one_set

N, A, K, B, P, HW = 8, 16368, 200, 20, 4, 128
HW2 = HW * HW  # 16384
KN = 3 * K  # 600
F32 = mybir.dt.float32
BF16 = mybir.dt.bfloat16
I32 = mybir.dt.int32
AF = mybir.ActivationFunctionType
ALU = mybir.AluOpType

# weights fold the final /8 mean over cores
W_POS = -1.0 / (K * K * N)        # stats hold +ln(p)
W_NEG = -1.0 / (KN * K * N)       # stats hold +ln(1-p)
W_LOC = 1.0 / (K * N)
W_S1 = 1.0 / (HW2 * K * N)
W_S2 = -W_S1
INV_LN10 = float(1.0 / np.log(10.0))
LN2 = float(np.log(2.0))
# deg-5 fit of ln(m) on [1,2), max abs err ~1e-5
LNC = [0.030449, -0.28382685, 1.11609003, -2.44002976, 3.5140873, -1.93675974]

ZSLOTS = 8
ZROUNDS = HW // ZSLOTS  # 16


def build_kernel():
    nc = bacc.Bacc(None, target_bir_lowering=False, debug=False)

    big = nc.declare_dram_parameter("big", [A, 13], F32, isOutput=False)
    cls = nc.declare_dram_parameter("cls", [A, 1], F32, isOutput=False)
    proto_z = nc.declare_dram_parameter("proto_z", [128, P * HW2 // 128], F32,
                                        isOutput=False)
    proto_g = nc.declare_dram_parameter("proto_g", [128, P * HW2 // 128], F32,
                                        isOutput=False)
    masks_g = nc.declare_dram_parameter("masks_g", [128, B * HW2 // 128], F32,
                                        isOutput=False)
    gtb = nc.declare_dram_parameter("gtb", [B, 4], F32, isOutput=False)
    # packed indices [128, 9]: c0 pos[:128], c1 pos[128:]+pad, c2 gt[:128],
    # c3 gt[128:]+pad, c4:9 neg (120 rows per col)
    idx = nc.declare_dram_parameter("idx", [128, 9], I32, isOutput=False)
    out = nc.declare_dram_parameter("out", [1, 1], F32, isOutput=True)

    PF_COLS = P * HW2 // 128  # 512
    MG_COLS = B * HW2 // 128  # 2560

    with tile.TileContext(nc) as tc:
        with tc.tile_pool(name="sb", bufs=1) as sb:
            # ---------------- constants / accumulators ----------------
            ident = sb.tile([128, 128], F32)
            make_identity(nc, ident[:])
            ones = sb.tile([128, 1], F32)
            nc.vector.memset(ones[:], 1.0)
            iota_i = sb.tile([128, B], I32)
            nc.gpsimd.iota(iota_i[:], pattern=[[1, B]], base=0,
                           channel_multiplier=0)
            iota_f = sb.tile([128, B], F32)
            nc.vector.tensor_copy(out=iota_f[:], in_=iota_i[:])
            stats = sb.tile([128, 8], F32)
            nc.vector.memset(stats[:], 0.0)
            macc = sb.tile([128, 4], F32)
            nc.vector.memset(macc[:], 0.0)
            # dummy exp so the act table loads during staging
            warm = sb.tile([1, 2], F32)
            nc.vector.memset(warm[:], 0.0)
            nc.scalar.activation(warm[0:1, 1:2], warm[0:1, 0:1], AF.Exp)

            # ---------------- small DMAs (sync queue, priority order) ------
            idxt = sb.tile([128, 9], I32)
            nc.sync.dma_start(out=idxt[:], in_=idx[:, :])
            pf = sb.tile([128, PF_COLS], F32)
            nc.sync.dma_start(out=pf[:], in_=proto_z[:, :])
            # G operands on the hardware DMA queue (the gpsimd software
            # queue moves ~11 GB/s and would starve the gathers)
            protoG = sb.tile([128, PF_COLS], F32)
            nc.sync.dma_start(out=protoG[:], in_=proto_g[:, :])
            masksG = sb.tile([128, MG_COLS], F32)
            nc.sync.dma_start(out=masksG[:], in_=masks_g[:, :])
            posi1 = idxt[:, 0:1]
            posi2 = idxt[0:72, 1:2]
            gti1 = idxt[:, 2:3]
            gti2 = idxt[0:72, 3:4]

            # ---------------- gathers (gpsimd; coef path first) ----------
            bigg1 = sb.tile([128, 13], F32)
            bigg2 = sb.tile([72, 13], F32)
            big_g1 = nc.gpsimd.indirect_dma_start(
                out=bigg1[:], out_offset=None, in_=big[:, :],
                in_offset=bacc.bass.IndirectOffsetOnAxis(ap=posi1, axis=0))
            big_g2 = nc.gpsimd.indirect_dma_start(
                out=bigg2[:], out_offset=None, in_=big[:, :],
                in_offset=bacc.bass.IndirectOffsetOnAxis(ap=posi2, axis=0))

            # ---------------- proto to bf16 [P, HW2] ----------------
            pb = sb.tile([128, PF_COLS], BF16)
            nc.vector.tensor_copy(out=pb[:], in_=pf[:])
            protoNb = sb.tile([P, HW2], BF16)
            nc.sync.dma_start(out=protoNb[:], in_=pb[:])

            # remaining gathers (behind bigg on the gpsimd queue)
            gtg1 = sb.tile([128, 4], F32)
            gtg2 = sb.tile([72, 4], F32)
            nc.gpsimd.indirect_dma_start(
                out=gtg1[:], out_offset=None, in_=gtb[:, :],
                in_offset=bacc.bass.IndirectOffsetOnAxis(ap=gti1, axis=0))
            nc.gpsimd.indirect_dma_start(
                out=gtg2[:], out_offset=None, in_=gtb[:, :],
                in_offset=bacc.bass.IndirectOffsetOnAxis(ap=gti2, axis=0))
            negp = sb.tile([120, 5], F32)
            for j in range(5):
                nc.gpsimd.indirect_dma_start(
                    out=negp[:, j:j + 1], out_offset=None, in_=cls[:, :],
                    in_offset=bacc.bass.IndirectOffsetOnAxis(
                        ap=idxt[0:120, 4 + j:5 + j], axis=0))


            # ---------------- coefT (critical path to z matmuls) ----------
            with tc.tile_pool(name="psA", bufs=1, space="PSUM") as psA:
                ctps = psA.tile([P, 256], F32)
                nc.tensor.transpose(out=ctps[:, 0:128], in_=bigg1[:, 4:8],
                                    identity=ident[:])
                nc.tensor.transpose(out=ctps[:, 128:200], in_=bigg2[:, 4:8],
                                    identity=ident[0:72, 0:72])
                coefTb = sb.tile([P, K], BF16)
                nc.scalar.copy(out=coefTb[:], in_=ctps[:, 0:200])

            # ---------------- bulk: z matmuls + exp/product-of-8/ln -------
            # sum softplus(z) = ln prod (1+e^z); products of 8 on DVE in
            # bf16, one batched Ln+accumulate per 4 rounds on ACT.
            scr = sb.tile([128, 8 * K], F32)
            z_mms = []
            sp_instrs = []
            with tc.tile_pool(name="psZ", bufs=2, space="PSUM") as psZ, \
                 tc.tile_pool(name="sb2", bufs=3) as sb2:
                vbuf = None
                for r in range(ZROUNDS):
                    zp = psZ.tile([128, ZSLOTS, 256], F32, tag="zp", name="zp")
                    for s in range(ZSLOTS):
                        t = r * ZSLOTS + s
                        mm = nc.tensor.matmul(
                            out=zp[:, s, 0:K],
                            lhsT=protoNb[:, t * 128:(t + 1) * 128],
                            rhs=coefTb[:], start=True, stop=True)
                        z_mms.append(mm)
                    if r % 4 == 0:
                        vbuf = sb2.tile([128, 8, K], BF16, tag="vbuf",
                                        name="vbuf")
                    et = sb2.tile([128, ZSLOTS * K], BF16, tag="et", name="et")
                    sp = nc.scalar.activation(et[:], zp[:, :, 0:K], AF.Exp)
                    sp_instrs.append(sp)
                    nc.vector.tensor_scalar_add(et[:], et[:], 1.0)
                    t8 = sb2.tile([128, 4 * K], BF16, tag="t8", name="t8")
                    nc.vector.tensor_tensor(out=t8[:], in0=et[:, 0:4 * K],
                                            in1=et[:, 4 * K:8 * K],
                                            op=ALU.mult)
                    nc.vector.tensor_tensor(
                        out=vbuf[:, 2 * (r % 4):2 * (r % 4) + 2, :],
                        in0=t8[:, 0:2 * K], in1=t8[:, 2 * K:4 * K],
                        op=ALU.mult)
                    if r % 4 == 3:
                        nc.scalar.activation(scr[:], vbuf[:], AF.Ln,
                                             accum_out=macc[:, r // 4:r // 4 + 1])

                # ---------------- small losses, all on DVE ----------------
                # lnL layout [128,15]: 0 p1 | 1:3 ahw1 | 3:5 gt1 | 5 p2 |
                #   6:8 ahw2 | 8:10 gt2 | 10:15 (1-pneg)
                lnL = sb.tile([128, 15], F32)
                small = []
                small.append(nc.vector.memset(lnL[:], 1.0))
                small.append(nc.vector.tensor_copy(out=lnL[:, 0:1],
                                                   in_=bigg1[:, 12:13]))
                small.append(nc.vector.tensor_copy(out=lnL[:, 1:3],
                                                   in_=bigg1[:, 10:12]))
                small.append(nc.vector.tensor_copy(out=lnL[:, 3:5],
                                                   in_=gtg1[:, 2:4]))
                small.append(nc.vector.tensor_copy(out=lnL[0:72, 5:6],
                                                   in_=bigg2[:, 12:13]))
                small.append(nc.vector.tensor_copy(out=lnL[0:72, 6:8],
                                                   in_=bigg2[:, 10:12]))
                small.append(nc.vector.tensor_copy(out=lnL[0:72, 8:10],
                                                   in_=gtg2[:, 2:4]))
                small.append(nc.vector.tensor_scalar(
                    out=lnL[0:120, 10:15], in0=negp[:], scalar1=-1.0,
                    scalar2=1.0, op0=ALU.mult, op1=ALU.add))
                # ln via exponent/mantissa split + Horner
                bits = lnL[:].bitcast(I32)
                eI = sb.tile([128, 15], I32)
                small.append(nc.vector.tensor_scalar(
                    out=eI[:], in0=bits, scalar1=23, scalar2=None,
                    op0=ALU.logical_shift_right))
                eF = sb.tile([128, 15], F32)
                small.append(nc.vector.tensor_copy(out=eF[:], in_=eI[:]))
                eT = sb.tile([128, 15], F32)
                small.append(nc.vector.tensor_scalar(
                    out=eT[:], in0=eF[:], scalar1=LN2, scalar2=-127.0 * LN2,
                    op0=ALU.mult, op1=ALU.add))
                mI = sb.tile([128, 15], I32)
                small.append(nc.vector.tensor_scalar(
                    out=mI[:], in0=bits, scalar1=0x007FFFFF, scalar2=0x3F800000,
                    op0=ALU.bitwise_and, op1=ALU.bitwise_or))
                mant = mI[:].bitcast(F32)
                h = sb.tile([128, 15], F32)
                small.append(nc.vector.tensor_scalar(
                    out=h[:], in0=mant, scalar1=LNC[0], scalar2=LNC[1],
                    op0=ALU.mult, op1=ALU.add))
                for c in LNC[2:]:
                    small.append(nc.vector.tensor_tensor(
                        out=h[:], in0=h[:], in1=mant, op=ALU.mult))
                    small.append(nc.vector.tensor_scalar_add(h[:], h[:], float(c)))
                small.append(nc.vector.tensor_tensor(
                    out=h[:], in0=h[:], in1=eT[:], op=ALU.add))
                # h now = ln of every lnL column
                # cls
                small.append(nc.vector.tensor_scalar(
                    out=stats[0:128, 0:1], in0=h[:, 0:1], scalar1=W_POS,
                    scalar2=None, op0=ALU.mult))
                small.append(nc.vector.tensor_scalar(
                    out=stats[0:72, 1:2], in0=h[0:72, 5:6], scalar1=W_POS,
                    scalar2=None, op0=ALU.mult))
                negred = sb.tile([120, 1], F32)
                small.append(nc.vector.tensor_reduce(
                    out=negred[:], in_=h[0:120, 10:15],
                    axis=mybir.AxisListType.X, op=ALU.add))
                small.append(nc.vector.tensor_scalar(
                    out=stats[0:120, 2:3], in0=negred[:], scalar1=W_NEG,
                    scalar2=None, op0=ALU.mult))
                # loc: big cols 0:4 pr, 8:10 ac, 10:12 ahw
                for ci, (bigg, gtg, q, acol, lo) in enumerate(
                        [(bigg1, gtg1, 128, 3, 0), (bigg2, gtg2, 72, 4, 5)]):
                    inv = sb.tile([128, 2], F32, tag=f"inv{ci}", name=f"inv{ci}")
                    small.append(nc.vector.reciprocal(inv[0:q, :], bigg[:, 10:12]))
                    d = sb.tile([128, 4], F32, tag=f"d{ci}", name=f"d{ci}")
                    small.append(nc.vector.tensor_tensor(
                        out=d[0:q, 0:2], in0=gtg[:, 0:2], in1=bigg[:, 8:10],
                        op=ALU.subtract))
                    small.append(nc.vector.tensor_tensor(
                        out=d[0:q, 0:2], in0=d[0:q, 0:2], in1=inv[0:q, :],
                        op=ALU.mult))
                    small.append(nc.vector.tensor_tensor(
                        out=d[0:q, 0:2], in0=bigg[:, 0:2], in1=d[0:q, 0:2],
                        op=ALU.subtract))
                    dln = sb.tile([128, 2], F32, tag=f"dln{ci}", name=f"dln{ci}")
                    small.append(nc.vector.tensor_tensor(
                        out=dln[0:q, :], in0=h[0:q, lo + 3:lo + 5],
                        in1=h[0:q, lo + 1:lo + 3], op=ALU.subtract))
                    small.append(nc.vector.tensor_scalar(
                        out=dln[0:q, :], in0=dln[0:q, :], scalar1=-INV_LN10,
                        scalar2=None, op0=ALU.mult))
                    small.append(nc.vector.tensor_tensor(
                        out=d[0:q, 2:4], in0=bigg[:, 2:4], in1=dln[0:q, :],
                        op=ALU.add))
                    nd = sb.tile([128, 4], F32, tag=f"nd{ci}", name=f"nd{ci}")
                    small.append(nc.vector.tensor_scalar(
                        out=nd[0:q, :], in0=d[0:q, :], scalar1=-1.0,
                        scalar2=None, op0=ALU.mult))
                    ad = sb.tile([128, 4], F32, tag=f"ad{ci}", name=f"ad{ci}")
                    small.append(nc.vector.tensor_tensor(
                        out=ad[0:q, :], in0=d[0:q, :], in1=nd[0:q, :],
                        op=ALU.max))
                    m = sb.tile([128, 4], F32, tag=f"m{ci}", name=f"m{ci}")
                    small.append(nc.vector.tensor_scalar(
                        out=m[0:q, :], in0=ad[0:q, :], scalar1=1.0,
                        scalar2=None, op0=ALU.min))
                    sq = sb.tile([128, 4], F32, tag=f"sq{ci}", name=f"sq{ci}")
                    small.append(nc.vector.tensor_tensor(
                        out=sq[0:q, :], in0=m[0:q, :], in1=m[0:q, :],
                        op=ALU.mult))
                    small.append(nc.vector.tensor_scalar(
                        out=sq[0:q, :], in0=sq[0:q, :], scalar1=0.5,
                        scalar2=None, op0=ALU.mult))
                    small.append(nc.vector.tensor_tensor(
                        out=ad[0:q, :], in0=ad[0:q, :], in1=m[0:q, :],
                        op=ALU.subtract))
                    small.append(nc.vector.tensor_tensor(
                        out=sq[0:q, :], in0=sq[0:q, :], in1=ad[0:q, :],
                        op=ALU.add))
                    red = sb.tile([128, 1], F32, tag=f"red{ci}", name=f"red{ci}")
                    small.append(nc.vector.tensor_reduce(
                        out=red[0:q, :], in_=sq[0:q, :],
                        axis=mybir.AxisListType.X, op=ALU.add))
                    small.append(nc.vector.tensor_scalar(
                        out=stats[0:q, acol:acol + 1], in0=red[0:q, :],
                        scalar1=W_LOC, scalar2=None, op0=ALU.mult))

                # keep the small DVE chain out of the bulk ramp-up
                for si in small:
                    tile.add_dep_helper(si.ins, sp_instrs[2].ins, sync=False,
                                        reason="smalls after bulk start")

                # ---------------- G = masks @ proto^T on PE ----------------
                # bf16 copies for single-pass G matmuls; pinned late so the
                # in-order DVE queue never blocks on the G loads mid-bulk
                protoGb = sb.tile([128, PF_COLS], BF16)
                pgc = nc.vector.tensor_copy(out=protoGb[:], in_=protoG[:])
                masksGb = sb.tile([128, MG_COLS], BF16)
                mgc = nc.vector.tensor_copy(out=masksGb[:], in_=masksG[:])
                tile.add_dep_helper(pgc.ins, sp_instrs[8].ins, sync=False,
                                    reason="G casts late on DVE")
                tile.add_dep_helper(mgc.ins, sp_instrs[8].ins, sync=False,
                                    reason="G casts late on DVE")
                Gps = psZ.tile([B, P], F32, tag="zp", name="Gps")
                g_mms = []
                for j in range(HW):
                    gm = nc.tensor.matmul(
                        out=Gps[:], lhsT=masksGb[:, j * B:(j + 1) * B],
                        rhs=protoGb[:, j * P:(j + 1) * P],
                        start=(j == 0), stop=(j == HW - 1))
                    g_mms.append(gm)
                tile.add_dep_helper(g_mms[0].ins, z_mms[-1].ins, sync=False,
                                    reason="G after z stream")

                # ---------------- c_agg then S2 = <c_agg, G> --------------
                gidx1 = sb.tile([128, 1], F32)
                gidx2 = sb.tile([72, 1], F32)
                nc.vector.tensor_copy(out=gidx1[:], in_=gti1)
                nc.vector.tensor_copy(out=gidx2[:], in_=gti2)
                H1 = sb.tile([128, B], F32)
                H2 = sb.tile([72, B], F32)
                nc.vector.tensor_scalar(out=H1[:], in0=iota_f[:],
                                        scalar1=gidx1[:, :1], scalar2=None,
                                        op0=ALU.is_equal)
                nc.vector.tensor_scalar(out=H2[:], in0=iota_f[0:72, :],
                                        scalar1=gidx2[:, :1], scalar2=None,
                                        op0=ALU.is_equal)
                cagg = psZ.tile([B, P], F32, tag="zp", name="cagg")
                cm1 = nc.tensor.matmul(out=cagg[:], lhsT=H1[:],
                                       rhs=bigg1[:, 4:8], start=True, stop=False)
                nc.tensor.matmul(out=cagg[:], lhsT=H2[:], rhs=bigg2[:, 4:8],
                                 start=False, stop=True)
                tile.add_dep_helper(cm1.ins, z_mms[-1].ins, sync=False,
                                    reason="cagg after z stream")
                cagg_sb = sb.tile([B, P], F32)
                nc.vector.tensor_copy(out=cagg_sb[:], in_=cagg[:])
                s2 = sb.tile([B, P], F32)
                nc.vector.tensor_tensor(out=s2[:], in0=cagg_sb[:], in1=Gps[:],
                                        op=ALU.mult)
                s2r = sb.tile([B, 1], F32)
                nc.vector.tensor_reduce(out=s2r[:], in_=s2[:],
                                        axis=mybir.AxisListType.X, op=ALU.add)
                nc.vector.tensor_scalar(out=stats[0:B, 5:6], in0=s2r[:],
                                        scalar1=W_S2, scalar2=None, op0=ALU.mult)

                # ---------------- final combine ----------------
                m1 = sb.tile([128, 1], F32)
                nc.vector.tensor_reduce(out=m1[:], in_=macc[:],
                                        axis=mybir.AxisListType.X, op=ALU.add)
                nc.vector.tensor_scalar(out=stats[:, 6:7], in0=m1[:],
                                        scalar1=W_S1, scalar2=None, op0=ALU.mult)
                total = sb.tile([128, 1], F32)
                nc.vector.tensor_reduce(out=total[:], in_=stats[:],
                                        axis=mybir.AxisListType.X, op=ALU.add)
                totps = psZ.tile([1, 1], F32, tag="zp", name="totps")
                nc.tensor.matmul(out=totps[:], lhsT=total[:], rhs=ones[:],
                                 start=True, stop=True)
                fin = sb.tile([1, 1], F32)
                nc.vector.tensor_copy(out=fin[:], in_=totps[0:1, 0:1])
                nc.sync.dma_start(out=out[:, :], in_=fin[:])

    nc.finalize()
    return nc


def _pack_idx(pos, neg, gt):
    m = np.zeros((128, 9), dtype=np.int32)
    m[:, 0] = pos[0:128]
    m[0:72, 1] = pos[128:200]
    m[:, 2] = gt[0:128]
    m[0:72, 3] = gt[128:200]
    for j in range(5):
        m[0:120, 4 + j] = neg[j * 120:(j + 1) * 120]
    return m


def make_in_maps(map_class, map_box, map_coef, proto, anchor_center, anchor_hw,
                 gt_boxes, gt_masks, pos_idx, neg_idx, gt_idx):
    in_maps = []
    for i in range(N):
        big = np.concatenate(
            [map_box[i], map_coef[i], anchor_center, anchor_hw,
             map_class[i].reshape(A, 1)], axis=1).astype(np.float32)
        in_maps.append(dict(
            big=np.ascontiguousarray(big),
            cls=np.ascontiguousarray(map_class[i].reshape(A, 1)),
            proto_z=np.ascontiguousarray(
                proto[i].reshape(128, P * HW2 // 128)),
            proto_g=np.ascontiguousarray(
                proto[i].transpose(2, 1, 0).reshape(128, P * HW2 // 128)),
            masks_g=np.ascontiguousarray(
                gt_masks[i].transpose(2, 1, 0).reshape(128, B * HW2 // 128)),
            gtb=np.ascontiguousarray(gt_boxes[i]),
            idx=_pack_idx(pos_idx[i], neg_idx[i], gt_idx[i]),
        ))
    return in_maps


def kernel(**inputs):
    from concourse.bass_utils import run_bass_kernel_spmd
    nc = build_kernel()
    in_maps = make_in_maps(**inputs)
    res = run_bass_kernel_spmd(nc, in_maps, core_ids=list(range(N)))
    return np.float32(sum(float(res.results[c]["out"][0, 0]) for c in range(N)))


# revision 19
# speedup vs baseline: 1.1292x; 1.0125x over previous
"""Distributed Bass kernel for nn_AllLoss: YOLACT-style loss over 8 cores.

Per-core (one image each):
  cls:  -ln(p[pos]).sum()/K/K  +  -ln(1-p[neg]).sum()/3K/K
  loc:  smooth_l1(pr - encode(gt, anchor)).sum()/K
  msk:  BCE(sigmoid(coef@proto), goalmask).mean(hw).sum(k)/K
        = [ sum softplus(z) - sum_k <y_gk, z_k> ] / 16384 / K
        with  sum_k <y_gk, z_k> = <c_agg, G>,  G[b,p] = <y_b, proto_p>,
              c_agg = onehot(gt_idx)^T @ coef_gathered

Engine plan:
  ACT: exp over z (16 rounds of [128,1600] from PSUM); scaled products of
       16 on DVE (each (1+e^z) scaled by e^-3.5 so 16-products stay in
       f32/bf16 range); one batched Ln+accumulate per 8 rounds.  The
       constant 3.5*K*HW2 scale deficit is added back at the end.
  PE:  G via 128 accumulated bf16 matmuls run BEFORE the z stream (PE is
       otherwise idle during DMA staging); then coefT transposes, c_agg,
       z = protoN^T @ coefT (128 bf16 matmuls), final partition-reduce.
  DVE: bulk products, all small losses.  ln() for the small losses uses an
       exponent/mantissa bit split + deg-5 Horner polynomial (no ACT table
       switch, table stays natural_log_exp_and_others).
  No collective: each core writes its per-image partial (already /N); the
  host sums the 8 scalars as the unshard step.
"""
import sys

sys.path.insert(0, "/opt/trn_rl_repo")
import numpy as np
from concourse import bacc, mybir, tile
from concourse.masks import make_identity

# Pin the single activation table we use (Exp + Ln live together here);
# left alone, the table-load pass may thrash between sets.
_orig_gat = bacc.get_activation_tables


def _gat_one_set(arch):
    t = _orig_gat(arch)
    keep = "natural_log_exp_and_others"
    return {k: (v if k == keep else set()) for k, v in t.items()}


bacc.get_activation_tables = _gat_one_set

N, A, K, B, P, HW = 8, 16368, 200, 20, 4, 128
HW2 = HW * HW  # 16384
KN = 3 * K  # 600
F32 = mybir.dt.float32
BF16 = mybir.dt.bfloat16
I32 = mybir.dt.int32
AF = mybir.ActivationFunctionType
ALU = mybir.AluOpType

# weights fold the final /8 mean over cores
W_POS = -1.0 / (K * K * N)        # stats hold +ln(p)
W_NEG = -1.0 / (KN * K * N)       # stats hold +ln(1-p)
W_LOC = 1.0 / (K * N)
W_S1 = 1.0 / (HW2 * K * N)
W_S2 = -W_S1
INV_LN10 = float(1.0 / np.log(10.0))
LN2 = float(np.log(2.0))
# deg-5 fit of ln(m) on [1,2), max abs err ~1e-5
LNC = [0.030449, -0.28382685, 1.11609003, -2.44002976, 3.5140873, -1.93675974]
SCALE1 = float(np.exp(-3.5))      # per-element scale on (1+e^z)
SCALE_FIX = 3.5 / N               # adds back 3.5*K*HW2*W_S1

ZSLOTS = 8
ZROUNDS = HW // ZSLOTS  # 16


def build_kernel():
    nc = bacc.Bacc(None, target_bir_lowering=False, debug=False)

    big = nc.declare_dram_parameter("big", [A, 13], F32, isOutput=False)
    cls = nc.declare_dram_parameter("cls", [A, 1], F32, isOutput=False)
    proto_z = nc.declare_dram_parameter("proto_z", [128, P * HW2 // 128], F32,
                                        isOutput=False)
    proto_g = nc.declare_dram_parameter("proto_g", [128, P * HW2 // 128], F32,
                                        isOutput=False)
    masks_g = nc.declare_dram_parameter("masks_g", [128, B * HW2 // 128], F32,
                                        isOutput=False)
    gtb = nc.declare_dram_parameter("gtb", [B, 4], F32, isOutput=False)
    # packed indices [128, 9]: c0 pos[:128], c1 pos[128:]+pad, c2 gt[:128],
    # c3 gt[128:]+pad, c4:9 neg (120 rows per col)
    idx = nc.declare_dram_parameter("idx", [128, 9], I32, isOutput=False)
    out = nc.declare_dram_parameter("out", [1, 1], F32, isOutput=True)

    PF_COLS = P * HW2 // 128  # 512
    MG_COLS = B * HW2 // 128  # 2560

    with tile.TileContext(nc) as tc:
        with tc.tile_pool(name="sb", bufs=1) as sb:
            # ---------------- constants / accumulators ----------------
            ident = sb.tile([128, 128], F32)
            make_identity(nc, ident[:])
            ones = sb.tile([128, 1], F32)
            nc.vector.memset(ones[:], 1.0)
            iota_i = sb.tile([128, B], I32)
            nc.gpsimd.iota(iota_i[:], pattern=[[1, B]], base=0,
                           channel_multiplier=0)
            iota_f = sb.tile([128, B], F32)
            nc.vector.tensor_copy(out=iota_f[:], in_=iota_i[:])
            stats = sb.tile([128, 8], F32)
            nc.vector.memset(stats[:], 0.0)
            nc.vector.memset(stats[0:1, 7:8], SCALE_FIX)
            macc = sb.tile([128, 2], F32)
            nc.vector.memset(macc[:], 0.0)
            # dummy exp so the act table loads during staging
            warm = sb.tile([1, 2], F32)
            nc.vector.memset(warm[:], 0.0)
            nc.scalar.activation(warm[0:1, 1:2], warm[0:1, 0:1], AF.Exp)

            # ---------------- DMAs (sync = hardware queue) ----------------
            idxt = sb.tile([128, 9], I32)
            nc.sync.dma_start(out=idxt[:], in_=idx[:, :])
            masksG = sb.tile([128, MG_COLS], F32)
            nc.sync.dma_start(out=masksG[:], in_=masks_g[:, :])
            pf = sb.tile([128, PF_COLS], F32)
            nc.sync.dma_start(out=pf[:], in_=proto_z[:, :])
            protoG = sb.tile([128, PF_COLS], F32)
            nc.sync.dma_start(out=protoG[:], in_=proto_g[:, :])
            posi1 = idxt[:, 0:1]
            posi2 = idxt[0:72, 1:2]
            gti1 = idxt[:, 2:3]
            gti2 = idxt[0:72, 3:4]

            # ---------------- gathers (gpsimd; coef path first) ----------
            bigg1 = sb.tile([128, 13], F32)
            bigg2 = sb.tile([72, 13], F32)
            nc.gpsimd.indirect_dma_start(
                out=bigg1[:], out_offset=None, in_=big[:, :],
                in_offset=bacc.bass.IndirectOffsetOnAxis(ap=posi1, axis=0))
            nc.gpsimd.indirect_dma_start(
                out=bigg2[:], out_offset=None, in_=big[:, :],
                in_offset=bacc.bass.IndirectOffsetOnAxis(ap=posi2, axis=0))

            # early bf16 casts (DVE is idle during staging)
            pb = sb.tile([128, PF_COLS], BF16)
            nc.vector.tensor_copy(out=pb[:], in_=pf[:])
            protoNb = sb.tile([P, HW2], BF16)
            nc.sync.dma_start(out=protoNb[:], in_=pb[:])
            protoGb = sb.tile([128, PF_COLS], BF16)
            nc.vector.tensor_copy(out=protoGb[:], in_=protoG[:])
            masksGb = sb.tile([128, MG_COLS], BF16)
            nc.vector.tensor_copy(out=masksGb[:], in_=masksG[:])

            # remaining gathers (behind bigg on the gpsimd queue)
            gtg1 = sb.tile([128, 4], F32)
            gtg2 = sb.tile([72, 4], F32)
            nc.gpsimd.indirect_dma_start(
                out=gtg1[:], out_offset=None, in_=gtb[:, :],
                in_offset=bacc.bass.IndirectOffsetOnAxis(ap=gti1, axis=0))
            nc.gpsimd.indirect_dma_start(
                out=gtg2[:], out_offset=None, in_=gtb[:, :],
                in_offset=bacc.bass.IndirectOffsetOnAxis(ap=gti2, axis=0))
            negp = sb.tile([120, 5], F32)
            for j in range(5):
                nc.gpsimd.indirect_dma_start(
                    out=negp[:, j:j + 1], out_offset=None, in_=cls[:, :],
                    in_offset=bacc.bass.IndirectOffsetOnAxis(
                        ap=idxt[0:120, 4 + j:5 + j], axis=0))

            # ---------- startup PE work: G, coefT, c_agg (pool psA) --------
            with tc.tile_pool(name="psA", bufs=1, space="PSUM") as psA:
                # G^T[p, b] = sum_pixels proto_p * y_b, accumulated over the
                # 128 column-chunks; runs entirely in the DMA-staging window
                GpsT = psA.tile([P, B], F32)
                for j in range(HW):
                    nc.tensor.matmul(
                        out=GpsT[:], lhsT=protoGb[:, j * P:(j + 1) * P],
                        rhs=masksGb[:, j * B:(j + 1) * B],
                        start=(j == 0), stop=(j == HW - 1))

                ctps = psA.tile([P, 256], F32)
                nc.tensor.transpose(out=ctps[:, 0:128], in_=bigg1[:, 4:8],
                                    identity=ident[:])
                nc.tensor.transpose(out=ctps[:, 128:200], in_=bigg2[:, 4:8],
                                    identity=ident[0:72, 0:72])
                coefTb = sb.tile([P, K], BF16)
                nc.scalar.copy(out=coefTb[:], in_=ctps[:, 0:200])

                # c_agg^T[p, b] = sum_k coef[k, p] * onehot[k, b]
                gidx1 = sb.tile([128, 1], F32)
                gidx2 = sb.tile([72, 1], F32)
                nc.vector.tensor_copy(out=gidx1[:], in_=gti1)
                nc.vector.tensor_copy(out=gidx2[:], in_=gti2)
                H1 = sb.tile([128, B], F32)
                H2 = sb.tile([72, B], F32)
                nc.vector.tensor_scalar(out=H1[:], in0=iota_f[:],
                                        scalar1=gidx1[:, :1], scalar2=None,
                                        op0=ALU.is_equal)
                nc.vector.tensor_scalar(out=H2[:], in0=iota_f[0:72, :],
                                        scalar1=gidx2[:, :1], scalar2=None,
                                        op0=ALU.is_equal)
                caggT = psA.tile([P, B], F32)
                nc.tensor.matmul(out=caggT[:], lhsT=bigg1[:, 4:8], rhs=H1[:],
                                 start=True, stop=False)
                nc.tensor.matmul(out=caggT[:], lhsT=bigg2[:, 4:8], rhs=H2[:],
                                 start=False, stop=True)
                caggT_sb = sb.tile([P, B], F32)
                nc.vector.tensor_copy(out=caggT_sb[:], in_=caggT[:])
                # S2 = <c_agg, G>, done here so nothing blocks the tail
                s2 = sb.tile([P, B], F32)
                nc.vector.tensor_tensor(out=s2[:], in0=caggT_sb[:],
                                        in1=GpsT[:], op=ALU.mult)
                s2r = sb.tile([P, 1], F32)
                nc.vector.tensor_reduce(out=s2r[:], in_=s2[:],
                                        axis=mybir.AxisListType.X, op=ALU.add)
                nc.vector.tensor_scalar(out=stats[0:P, 5:6], in0=s2r[:],
                                        scalar1=W_S2, scalar2=None,
                                        op0=ALU.mult)

            # ---------------- bulk: z matmuls + exp/product-of-16/ln ------
            # sum softplus(z) = ln prod (1+e^z); each factor scaled by
            # e^-3.5 so products of 16 stay within f32/bf16 range; one
            # batched Ln+accumulate per 8 rounds on ACT.
            scr = sb.tile([128, 8 * K], F32)
            z_mms = []
            sp_instrs = []
            with tc.tile_pool(name="psZ", bufs=2, space="PSUM") as psZ, \
                 tc.tile_pool(name="sb2", bufs=3) as sb2:
                vbuf = None
                t8_prev = None
                for r in range(ZROUNDS):
                    zp = psZ.tile([128, ZSLOTS, 256], F32, tag="zp", name="zp")
                    for s in range(ZSLOTS):
                        t = r * ZSLOTS + s
                        mm = nc.tensor.matmul(
                            out=zp[:, s, 0:K],
                            lhsT=protoNb[:, t * 128:(t + 1) * 128],
                            rhs=coefTb[:], start=True, stop=True)
                        z_mms.append(mm)
                    if r % 8 == 0:
                        vbuf = sb2.tile([128, 8, K], BF16, tag="vbuf",
                                        name="vbuf")
                    et = sb2.tile([128, ZSLOTS * K], BF16, tag="et", name="et")
                    sp = nc.scalar.activation(et[:], zp[:, :, 0:K], AF.Exp)
                    sp_instrs.append(sp)
                    # (1 + e^z) * e^-3.5
                    nc.vector.tensor_scalar(out=et[:], in0=et[:], scalar1=1.0,
                                            scalar2=SCALE1, op0=ALU.add,
                                            op1=ALU.mult)
                    t8 = sb2.tile([128, 4 * K], BF16, tag="t8", name="t8")
                    nc.vector.tensor_tensor(out=t8[:], in0=et[:, 0:4 * K],
                                            in1=et[:, 4 * K:8 * K],
                                            op=ALU.mult)
                    if r % 2 == 0:
                        t8_prev = t8
                    else:
                        v8 = sb2.tile([128, 4 * K], BF16, tag="v8", name="v8")
                        nc.vector.tensor_tensor(out=v8[:], in0=t8_prev[:],
                                                in1=t8[:], op=ALU.mult)
                        m = (r % 8) // 2
                        nc.vector.tensor_tensor(
                            out=vbuf[:, 2 * m:2 * m + 2, :],
                            in0=v8[:, 0:2 * K], in1=v8[:, 2 * K:4 * K],
                            op=ALU.mult)
                    if r % 8 == 7:
                        nc.scalar.activation(scr[:], vbuf[:], AF.Ln,
                                             accum_out=macc[:, r // 8:r // 8 + 1])

                # ---------------- small losses, all on DVE ----------------
                # lnL layout [128,15]: 0 p1 | 1:3 ahw1 | 3:5 gt1 | 5 p2 |
                #   6:8 ahw2 | 8:10 gt2 | 10:15 (1-pneg)
                lnL = sb.tile([128, 15], F32)
                small = []
                small.append(nc.vector.memset(lnL[:], 1.0))
                small.append(nc.vector.tensor_copy(out=lnL[:, 0:1],
                                                   in_=bigg1[:, 12:13]))
                small.append(nc.vector.tensor_copy(out=lnL[:, 1:3],
                                                   in_=bigg1[:, 10:12]))
                small.append(nc.vector.tensor_copy(out=lnL[:, 3:5],
                                                   in_=gtg1[:, 2:4]))
                small.append(nc.vector.tensor_copy(out=lnL[0:72, 5:6],
                                                   in_=bigg2[:, 12:13]))
                small.append(nc.vector.tensor_copy(out=lnL[0:72, 6:8],
                                                   in_=bigg2[:, 10:12]))
                small.append(nc.vector.tensor_copy(out=lnL[0:72, 8:10],
                                                   in_=gtg2[:, 2:4]))
                small.append(nc.vector.tensor_scalar(
                    out=lnL[0:120, 10:15], in0=negp[:], scalar1=-1.0,
                    scalar2=1.0, op0=ALU.mult, op1=ALU.add))
                # ln via exponent/mantissa split + Horner
                bits = lnL[:].bitcast(I32)
                eI = sb.tile([128, 15], I32)
                small.append(nc.vector.tensor_scalar(
                    out=eI[:], in0=bits, scalar1=23, scalar2=None,
                    op0=ALU.logical_shift_right))
                eF = sb.tile([128, 15], F32)
                small.append(nc.vector.tensor_copy(out=eF[:], in_=eI[:]))
                eT = sb.tile([128, 15], F32)
                small.append(nc.vector.tensor_scalar(
                    out=eT[:], in0=eF[:], scalar1=LN2, scalar2=-127.0 * LN2,
                    op0=ALU.mult, op1=ALU.add))
                mI = sb.tile([128, 15], I32)
                small.append(nc.vector.tensor_scalar(
                    out=mI[:], in0=bits, scalar1=0x007FFFFF, scalar2=0x3F800000,
                    op0=ALU.bitwise_and, op1=ALU.bitwise_or))
                mant = mI[:].bitcast(F32)
                h = sb.tile([128, 15], F32)
                small.append(nc.vector.tensor_scalar(
                    out=h[:], in0=mant, scalar1=LNC[0], scalar2=LNC[1],
                    op0=ALU.mult, op1=ALU.add))
                for c in LNC[2:]:
                    small.append(nc.vector.tensor_tensor(
                        out=h[:], in0=h[:], in1=mant, op=ALU.mult))
                    small.append(nc.vector.tensor_scalar_add(h[:], h[:], float(c)))
                small.append(nc.vector.tensor_tensor(
                    out=h[:], in0=h[:], in1=eT[:], op=ALU.add))
                # h now = ln of every lnL column
                # cls
                small.append(nc.vector.tensor_scalar(
                    out=stats[0:128, 0:1], in0=h[:, 0:1], scalar1=W_POS,
                    scalar2=None, op0=ALU.mult))
                small.append(nc.vector.tensor_scalar(
                    out=stats[0:72, 1:2], in0=h[0:72, 5:6], scalar1=W_POS,
                    scalar2=None, op0=ALU.mult))
                negred = sb.tile([120, 1], F32)
                small.append(nc.vector.tensor_reduce(
                    out=negred[:], in_=h[0:120, 10:15],
                    axis=mybir.AxisListType.X, op=ALU.add))
                small.append(nc.vector.tensor_scalar(
                    out=stats[0:120, 2:3], in0=negred[:], scalar1=W_NEG,
                    scalar2=None, op0=ALU.mult))
                # loc: big cols 0:4 pr, 8:10 ac, 10:12 ahw
                for ci, (bigg, gtg, q, acol, lo) in enumerate(
                        [(bigg1, gtg1, 128, 3, 0), (bigg2, gtg2, 72, 4, 5)]):
                    inv = sb.tile([128, 2], F32, tag=f"inv{ci}", name=f"inv{ci}")
                    small.append(nc.vector.reciprocal(inv[0:q, :], bigg[:, 10:12]))
                    d = sb.tile([128, 4], F32, tag=f"d{ci}", name=f"d{ci}")
                    small.append(nc.vector.tensor_tensor(
                        out=d[0:q, 0:2], in0=gtg[:, 0:2], in1=bigg[:, 8:10],
                        op=ALU.subtract))
                    small.append(nc.vector.tensor_tensor(
                        out=d[0:q, 0:2], in0=d[0:q, 0:2], in1=inv[0:q, :],
                        op=ALU.mult))
                    small.append(nc.vector.tensor_tensor(
                        out=d[0:q, 0:2], in0=bigg[:, 0:2], in1=d[0:q, 0:2],
                        op=ALU.subtract))
                    dln = sb.tile([128, 2], F32, tag=f"dln{ci}", name=f"dln{ci}")
                    small.append(nc.vector.tensor_tensor(
                        out=dln[0:q, :], in0=h[0:q, lo + 3:lo + 5],
                        in1=h[0:q, lo + 1:lo + 3], op=ALU.subtract))
                    small.append(nc.vector.tensor_scalar(
                        out=dln[0:q, :], in0=dln[0:q, :], scalar1=-INV_LN10,
                        scalar2=None, op0=ALU.mult))
                    small.append(nc.vector.tensor_tensor(
                        out=d[0:q, 2:4], in0=bigg[:, 2:4], in1=dln[0:q, :],
                        op=ALU.add))
                    nd = sb.tile([128, 4], F32, tag=f"nd{ci}", name=f"nd{ci}")
                    small.append(nc.vector.tensor_scalar(
                        out=nd[0:q, :], in0=d[0:q, :], scalar1=-1.0,
                        scalar2=None, op0=ALU.mult))
                    ad = sb.tile([128, 4], F32, tag=f"ad{ci}", name=f"ad{ci}")
                    small.append(nc.vector.tensor_tensor(
                        out=ad[0:q, :], in0=d[0:q, :], in1=nd[0:q, :],
                        op=ALU.max))
                    m = sb.tile([128, 4], F32, tag=f"m{ci}", name=f"m{ci}")
                    small.append(nc.vector.tensor_scalar(
                        out=m[0:q, :], in0=ad[0:q, :], scalar1=1.0,
                        scalar2=None, op0=ALU.min))
                    sq = sb.tile([128, 4], F32, tag=f"sq{ci}", name=f"sq{ci}")
                    small.append(nc.vector.tensor_tensor(
                        out=sq[0:q, :], in0=m[0:q, :], in1=m[0:q, :],
                        op=ALU.mult))
                    small.append(nc.vector.tensor_scalar(
                        out=sq[0:q, :], in0=sq[0:q, :], scalar1=0.5,
                        scalar2=None, op0=ALU.mult))
                    small.append(nc.vector.tensor_tensor(
                        out=ad[0:q, :], in0=ad[0:q, :], in1=m[0:q, :],
                        op=ALU.subtract))
                    small.append(nc.vector.tensor_tensor(
                        out=sq[0:q, :], in0=sq[0:q, :], in1=ad[0:q, :],
                        op=ALU.add))
                    red = sb.tile([128, 1], F32, tag=f"red{ci}", name=f"red{ci}")
                    small.append(nc.vector.tensor_reduce(
                        out=red[0:q, :], in_=sq[0:q, :],
                        axis=mybir.AxisListType.X, op=ALU.add))
                    small.append(nc.vector.tensor_scalar(
                        out=stats[0:q, acol:acol + 1], in0=red[0:q, :],
                        scalar1=W_LOC, scalar2=None, op0=ALU.mult))

                # keep the small DVE chain out of the bulk ramp-up
                for si in small:
                    tile.add_dep_helper(si.ins, sp_instrs[2].ins, sync=False,
                                        reason="smalls after bulk start")

                # ---------------- final combine ----------------
                m1 = sb.tile([128, 1], F32)
                nc.vector.tensor_reduce(out=m1[:], in_=macc[:],
                                        axis=mybir.AxisListType.X, op=ALU.add)
                nc.vector.tensor_scalar(out=stats[:, 6:7], in0=m1[:],
                                        scalar1=W_S1, scalar2=None, op0=ALU.mult)
                total = sb.tile([128, 1], F32)
                nc.vector.tensor_reduce(out=total[:], in_=stats[:],
                                        axis=mybir.AxisListType.X, op=ALU.add)
                totps = psZ.tile([1, 1], F32, tag="zp", name="totps")
                nc.tensor.matmul(out=totps[:], lhsT=total[:], rhs=ones[:],
                                 start=True, stop=True)
                fin = sb.tile([1, 1], F32)
                nc.vector.tensor_copy(out=fin[:], in_=totps[0:1, 0:1])
                nc.sync.dma_start(out=out[:, :], in_=fin[:])

    nc.finalize()
    return nc


def _pack_idx(pos, neg, gt):
    m = np.zeros((128, 9), dtype=np.int32)
    m[:, 0] = pos[0:128]
    m[0:72, 1] = pos[128:200]
    m[:, 2] = gt[0:128]
    m[0:72, 3] = gt[128:200]
    for j in range(5):
        m[0:120, 4 + j] = neg[j * 120:(j + 1) * 120]
    return m


def make_in_maps(map_class, map_box, map_coef, proto, anchor_center, anchor_hw,
                 gt_boxes, gt_masks, pos_idx, neg_idx, gt_idx):
    in_maps = []
    for i in range(N):
        big = np.concatenate(
            [map_box[i], map_coef[i], anchor_center, anchor_hw,
             map_class[i].reshape(A, 1)], axis=1).astype(np.float32)
        in_maps.append(dict(
            big=np.ascontiguousarray(big),
            cls=np.ascontiguousarray(map_class[i].reshape(A, 1)),
            proto_z=np.ascontiguousarray(
                proto[i].reshape(128, P * HW2 // 128)),
            proto_g=np.ascontiguousarray(
                proto[i].transpose(2, 1, 0).reshape(128, P * HW2 // 128)),
            masks_g=np.ascontiguousarray(
                gt_masks[i].transpose(2, 1, 0).reshape(128, B * HW2 // 128)),
            gtb=np.ascontiguousarray(gt_boxes[i]),
            idx=_pack_idx(pos_idx[i], neg_idx[i], gt_idx[i]),
        ))
    return in_maps


def kernel(**inputs):
    from concourse.bass_utils import run_bass_kernel_spmd
    nc = build_kernel()
    in_maps = make_in_maps(**inputs)
    res = run_bass_kernel_spmd(nc, in_maps, core_ids=list(range(N)))
    return np.float32(sum(float(res.results[c]["out"][0, 0]) for c in range(N)))
